# revision 2
# baseline (speedup 1.0000x reference)
"""Trainium2 Bass kernel for nn_LossTDSurv (survival loss over hazards).

Layout: each core owns 8 full idx-groups (one per "slot"; widths paired so
every core gets the same slot shapes -> one SPMD NEFF).  The host sends
q = 1-h in bf16, but per group only the columns the loss actually reads:

  slot s (group v, v in [8s, 8s+8)): ragged block [128, JB, V_s] holding
  q_0..q_{v-2} (padded with 1.0) -> a pairwise-multiply tree (2x bf16
  tensor_tensor) + one tensor_reduce(op=mult) per slot gives
  P2 = prod q_k = exp(cond_sum) directly: no bulk Ln, no Exp, no gathers.

Per-row aux arrays (q_v, q_{v-1}, event-masked h_v / q_v, events) let the
ACT engine produce five of the seven global sums for free via accum_out;
the two event-weighted sums use one scalar_tensor_tensor per piece.
ACT Ln saturates below ~3e-20 (HW-measured) so A = Ln(P2*2^50 + 1.1e-18)
and the host subtracts the 50*ln2 offsets.

Per-core output: [128, 24] f32 partial sums; host combines in float64:
   loss = 0.5*L_z + 0.5*L_c + 1.0*nll
"""

import numpy as np
import ml_dtypes

B_TOTAL = 524288
T = 64
N_CORES = 8
JB = 66                 # 128-row blocks per slot
GR = 128 * JB           # rows per slot = 8448
NSLOT = 8
# product width per slot: >= (max v in slot) - 1, padded so tree halves
# stay even down to the chosen depth
VW = [6, 14, 22, 32, 40, 48, 56, 64]
DEPTH = [0, 1, 1, 2, 2, 3, 3, 3]
FW = sum(JB * v for v in VW)             # ragged free width per partition
AW = NSLOT * JB                          # aux width per partition (528)
# epilogue pieces (slot ranges): s5/s6 arrive last -> keep their piece small
PIECES = [(0, 5), (7, 8), (5, 7)]

bf16 = ml_dtypes.bfloat16

_CACHE = {}


def _group_of(core, slot):
    return 8 * slot + (core if slot % 2 == 0 else 7 - core)


def _build_nc():
    """Single-core SPMD Bass program (same NEFF on all 8 cores)."""
    import concourse.bacc as bacc
    import concourse.mybir as mybir
    import concourse.tile as tile

    f32 = mybir.dt.float32
    bft = mybir.dt.bfloat16
    AF = mybir.ActivationFunctionType
    OP = mybir.AluOpType
    AX = mybir.AxisListType

    nc = bacc.Bacc("TRN2", target_bir_lowering=False, debug=False)

    qrag = nc.dram_tensor("qrag", [128, FW], bft, kind="ExternalInput")
    aux = nc.dram_tensor("aux", [128, 5 * AW], bft, kind="ExternalInput")
    partials = nc.dram_tensor("partials", [128, 24], f32, kind="ExternalOutput")

    # sync HWDGE ring: slots 0-6 in four chunks (first tiny -> DVE
    # starts ASAP); scalar HWDGE ring: slot 7 (arrives early, computed
    # right after s0; costs one ACT table re-load, off-critical); gpsimd
    # SWDGE ring: aux.
    SYNC_CHUNKS = [(0,), (1, 2), (3, 4), (5, 6)]
    SCALAR_CHUNKS = [(7,)]
    COMPUTE_ORDER = [0, 7, 1, 2, 3, 4, 5, 6]

    offs = {}
    off = 0
    for s in range(NSLOT):
        offs[s] = off
        off += JB * VW[s]

    with tile.TileContext(nc) as tc:
        with tc.tile_pool(name="pers", bufs=1) as pers:
            mt = {}

            def load_chunks(chunks, eng, pre):
                for gi, grp in enumerate(chunks):
                    width = sum(JB * VW[s] for s in grp)
                    t = pers.tile([128, width], bft, tag=f"{pre}{gi}",
                                  name=f"{pre}{gi}")
                    base = offs[grp[0]]
                    for s in grp:
                        mt[s] = (t, offs[s] - base)
                    eng.dma_start(t[:], qrag[:, base : base + width])

            load_chunks(SYNC_CHUNKS, nc.sync, "m")
            load_chunks(SCALAR_CHUNKS, nc.scalar, "sc")
            Aux = pers.tile([128, 5 * AW], bft, tag="aux")
            nc.gpsimd.dma_start(Aux[:], aux[:])
            Eb = Aux[:, 4 * AW : 5 * AW]

            acc = pers.tile([128, 24], f32, tag="acc")
            P2 = pers.tile([128, AW], f32, tag="P2")

            for s in COMPUTE_ORDER:
                tile_, o = mt[s]
                w = VW[s]
                cur = tile_[:, o : o + JB * w].rearrange(
                    "p (j w) -> p j w", w=w
                )
                for lvl in range(DEPTH[s]):
                    h = w // 2
                    nt = pers.tile([128, JB * h], bft, tag=f"t{s}_{lvl}",
                                   name=f"t{s}_{lvl}")
                    nv = nt[:].rearrange("p (j h) -> p j h", h=h)
                    nc.vector.tensor_tensor(
                        out=nv, in0=cur[:, :, 0:h], in1=cur[:, :, h:w],
                        op=OP.mult,
                    )
                    cur, w = nv, h
                nc.vector.tensor_reduce(
                    P2[:, s * JB : (s + 1) * JB], cur, axis=AX.X, op=OP.mult
                )

            junk = pers.tile([128, AW], f32, tag="junk")
            # free row-sums from ACT: cols 1..4 = sum ln Qv, ln Qv1, ln Hvm, ln Qvm
            for i in range(4):
                nc.scalar.activation(
                    junk[:], Aux[:, i * AW : (i + 1) * AW], AF.Ln,
                    accum_out=acc[:, 1 + i : 2 + i],
                )

            # epilogue in three pieces; the last-arriving slots (5,6) form
            # the final small piece so the post-reduce tail stays short
            Ab = pers.tile([128, AW], bft, tag="Ab")
            WQ = pers.tile([128, AW], f32, tag="WQ")
            LW = pers.tile([128, AW], bft, tag="LW")
            junk2 = pers.tile([128, AW], bft, tag="junk2")
            b_tiny = pers.tile([128, 1], f32, tag="b_tiny")
            b_eps = pers.tile([128, 1], f32, tag="b_eps")
            nc.gpsimd.memset(b_tiny[:], 1.1e-18)
            nc.gpsimd.memset(b_eps[:], 1e-8)

            for pi, (slo, shi) in enumerate(PIECES):
                a0 = 8 * pi
                sl = slice(slo * JB, shi * JB)
                nc.scalar.activation(Ab[:, sl], P2[:, sl], AF.Ln,
                                     scale=float(2.0 ** 50), bias=b_tiny[:],
                                     accum_out=acc[:, a0 : a0 + 1])
                nc.scalar.activation(WQ[:, sl], P2[:, sl], AF.Copy,
                                     scale=-1.0, bias=1.0)
                nc.scalar.activation(LW[:, sl], WQ[:, sl], AF.Ln,
                                     bias=b_eps[:])
                nc.vector.scalar_tensor_tensor(
                    out=junk2[:, sl], in0=Ab[:, sl], scalar=0.0,
                    in1=Eb[:, sl], op0=OP.add, op1=OP.mult,
                    accum_out=acc[:, a0 + 5 : a0 + 6],
                )
                nc.vector.scalar_tensor_tensor(
                    out=junk2[:, sl], in0=LW[:, sl], scalar=0.0,
                    in1=Eb[:, sl], op0=OP.add, op1=OP.mult,
                    accum_out=acc[:, a0 + 6 : a0 + 7],
                )

            nc.sync.dma_start(partials[:], acc[:])

    nc.finalize()
    return nc


def _pack_inputs(preds, target):
    """Returns per-core input maps, total event count and exact f64 sums for
    any overflow rows (normally none)."""
    preds = np.asarray(preds, np.float32).reshape(B_TOTAL, T)
    target = np.asarray(target, np.float32).reshape(B_TOTAL, 3)
    idx = target[:, 0].astype(np.int64)
    ev = target[:, 1].astype(np.float32)

    order = np.argsort(idx, kind="stable")
    idx_s = idx[order]
    starts = np.searchsorted(idx_s, np.arange(T + 1))
    h_s = preds[order]          # [B, T] f32 sorted by idx
    e_s = ev[order]

    n_events = float(ev.sum())
    n_events_dev = n_events          # events on device (minus overflow rows)
    extra = np.zeros(7, np.float64)
    in_maps = []
    for c in range(N_CORES):
        qragA = np.ones((128, FW), dtype=bf16)
        auxA = np.ones((128, 5 * AW), dtype=bf16)
        auxA[:, 4 * AW :] = 0
        off = 0
        for s in range(NSLOT):
            v = _group_of(c, s)
            w = VW[s]
            r0, r1 = int(starts[v]), int(starts[v + 1])
            n = r1 - r0
            n_dev = min(n, GR)
            h = h_s[r0 : r0 + n_dev]
            e = e_s[r0 : r0 + n_dev]
            q = 1.0 - h
            if v >= 2:
                blk = np.ones((GR, w), np.float32)
                blk[:n_dev, : v - 1] = q[:, : v - 1]
                qragA[:, off : off + JB * w] = (
                    blk.reshape(128, JB * w).astype(bf16)
                )
            off += JB * w
            a0 = s * JB

            def put(base, vals):
                col = np.ones(GR, np.float32)
                col[:n_dev] = vals
                auxA[:, base + a0 : base + a0 + JB] = (
                    col.reshape(128, JB).astype(bf16)
                )

            put(0, q[:, v])                                   # Qv
            put(AW, q[:, v - 1] if v >= 1 else np.ones(n_dev, np.float32))
            put(2 * AW, np.where(e > 0, h[:, v], 1.0))        # Hvm
            put(3 * AW, np.where(e > 0, q[:, v], 1.0))        # Qvm
            ecol = np.zeros(GR, np.float32)
            ecol[:n_dev] = e
            auxA[:, 4 * AW + a0 : 4 * AW + a0 + JB] = (
                ecol.reshape(128, JB).astype(bf16)
            )

            if n > GR:  # exact host fallback, normally unreachable
                ro = order[r0 + GR : r1]
                n_events_dev -= float(ev[ro].sum())
                h_o = preds[ro].astype(np.float64)
                q_o = 1.0 - h_o
                e_o = ev[ro].astype(np.float64)
                lg = np.log(q_o)
                A = lg[:, : v - 1].sum(axis=1) if v >= 2 else np.zeros(len(ro))
                lgv = lg[:, v]
                lgv1 = lg[:, v - 1] if v >= 1 else np.zeros(len(ro))
                loghv = np.log(h_o[:, v])
                logwt = np.log(np.clip(1.0 - np.exp(A), 1e-8, None))
                extra += [A.sum(), lgv.sum(), lgv1.sum(),
                          (e_o * loghv).sum(), (e_o * lgv).sum(),
                          (e_o * A).sum(), (e_o * logwt).sum()]

        in_maps.append({"qrag": qragA, "aux": auxA})
    return in_maps, n_events, n_events_dev, extra


def _combine(partials_list, n_events, n_events_dev, extra, b_total):
    s = np.zeros(7, np.float64)
    for p in partials_list:
        cols = p.astype(np.float64).sum(axis=0)
        s[0] += cols[0] + cols[8] + cols[16]      # sum A' (three pieces)
        s[1:5] += cols[1:5]
        s[5] += cols[5] + cols[13] + cols[21]     # sum e*A'
        s[6] += cols[6] + cols[14] + cols[22]     # sum e*logwt
    # undo the 2^50 pre-scale inside Ln(P2 * 2^50)
    ln2_50 = 50.0 * np.log(2.0)
    s[0] -= ln2_50 * (AW * 128) * len(partials_list)
    s[5] -= ln2_50 * n_events_dev
    s += extra
    sA, slnQv, slnQv1, slnHvm, slnQvm, seA, selw = s
    L_z = -(slnHvm + seA) / n_events
    L_c = -(sA + (selw - seA)) / b_total
    nll = -((sA + slnQv + slnQv1) + (slnHvm - slnQvm)) / b_total
    return np.float32(0.5 * L_z + 0.5 * L_c + 1.0 * nll)


def kernel(preds: np.ndarray, target: np.ndarray) -> np.ndarray:
    from concourse.bass_utils import run_bass_kernel_spmd

    if "nc" not in _CACHE:
        _CACHE["nc"] = _build_nc()
    nc = _CACHE["nc"]

    in_maps, n_events, n_events_dev, extra = _pack_inputs(preds, target)
    res = run_bass_kernel_spmd(nc, in_maps, core_ids=list(range(N_CORES)))
    _CACHE["last_results"] = res
    return _combine(
        [r["partials"] for r in res.results], n_events, n_events_dev, extra,
        float(B_TOTAL),
    )


if __name__ == "__main__":
    pass
